# revision 23
# baseline (speedup 1.0000x reference)
"""Trainium2 Bass kernel for nn_BoundaryDecision (sparse attention with scalar V).

Math: out = sigmoid(mask_last_row(  sum_n softmax_k(mask(q_n . k_n / sqrt(d)))  @ v_n ))
Key identity used: per-head V dim is 1, so we never materialize prob:
    attended_n[q] = A_n[q] / Z_n[q]
    Z_n[q] = sum_k maskc[q,k] * exp(s_n[q,k])
    A_n[q] = sum_k maskc[q,k] * exp(s_n[q,k]) * v_n[k]
Both are PE contractions over k of the masked escore^T tensor.

Sharding (8 cores): core c -> batch b=c//2, head-group g=c%2 (8 heads each).
Each core returns Z,A per (head, q); host does A/Z, head-sum across the two
head-group cores, final padded-mask + sigmoid.

v5 design (engine-balanced against the TimelineSim cost model, which matches
HW within ~2%; PE matmuls cost out_cols cycles SERIALLY -- tile_position
concurrency is NOT real on this target):
  - exp work split per k-tile: head A (1024 q) + head B q[512:1024] on ACT
    (true exp, fp8e5 out); head B q[0:512] on DVE as Schraudolph: scores
    pre-scaled by A8=4/ln2 (folded into Wq), tensor_scalar adds B8=59.8 and
    the int8 RNE convert yields e5m2 bits ~= exp(s) (HW-verified).
  - mask: one int16 bitwise-AND per k-tile over the fp8 esc pair (2x DVE
    mode; host packs {0x0000,0x00FF,0xFF00,0xFFFF} per q-pair).
  - Z/A: fp8 DoubleRow matmuls, contraction 256 = two k-tiles per stream
    (0.5 cyc/out-col), esc pair tiles [P, 2(kt parity), 2(head), 1024].
    DR outputs must sit at PSUM partition 0 -> four single-bank
    accumulators; evacuated by DMA straight from PSUM to DRAM.
  - proj: fp8e4 DoubleRow (x/16 and W*16 to stay in e4m3 normal range),
    4 x 256-contraction matmuls per 512-col tile, interleaved into the
    previous head-pair's k-loop.  Score matmuls stay fp16.
  - Z/A matmuls of a pair go behind the next scores in the PE FIFO so they
    never gate them (strict in-order engines).
"""

import os

import numpy as np

NEG = -60000.0
P = 128
QS = 2048
HID = 1024
N_HEADS = 16
HEAD_DIM = 64
NCORES = 8
HPC = 8  # heads per core

A8 = 8.0 / float(np.log(2.0))  # 11.5416 (Schraudolph e4m3 scale)
C0 = 4.0  # ACT-region exp shift: esc stores exp(s-4) in e4m3 (ceiling s=9.5)
C0B = 2.0  # Schraudolph-region shift (e5m2, ceiling s=13)
A5 = 4.0 / float(np.log(2.0))  # e5m2 Schraudolph scale
SCH_B5 = 59.8 - 2.0 * 4.0 / float(np.log(2.0))  # e5m2 offset w/ C0B shift
SCH_Q = 192  # q-columns per 512-q window handled by the DVE Schraudolph path
XSCALE = 16.0  # x/16, W*16 keeps fp8e4 operands in the normal range

_CACHE = {}


def _build_bass(loop_iters=1):
    import concourse.bass as bass
    import concourse.mybir as mybir
    from concourse import bacc, tile

    fp16 = mybir.dt.float16
    f32 = mybir.dt.float32
    f8e4 = mybir.dt.float8e4
    f8e5 = mybir.dt.float8e5
    i16 = mybir.dt.int16
    ts = bass.ts

    nc = bacc.Bacc(trn_type="TRN2")

    xT = nc.declare_dram_parameter("xT", [P, 8, QS], f8e4, isOutput=False)
    w = nc.declare_dram_parameter("w", [P, 8, 1040], f8e4, isOutput=False)
    maskT16 = nc.declare_dram_parameter("maskT16", [P, 16, QS // 2], i16, isOutput=False)
    za = nc.declare_dram_parameter("za", [2, HPC, QS], f32, isOutput=True)

    with tile.TileContext(nc) as tc:
        with (
            tc.tile_pool(name="big", bufs=1) as big,
            tc.tile_pool(name="work", bufs=3) as work,
            tc.tile_pool(name="psum", bufs=1, space="PSUM") as pp,
            tc.tile_pool(name="psum_za", bufs=1, space="PSUM") as pz,
        ):
            xT_sb = big.tile([P, 8, QS], f8e4, tag="xT")
            w_sb = big.tile([P, 8, 1040], f8e4, tag="w")
            # one tile per 2-k-tile chunk so the first mask consumer only
            # waits on its own chunk's DMA
            mask_sb = [
                big.tile([P, 2, QS // 2], i16, tag=f"mask{c}", name=f"mask{c}")
                for c in range(8)
            ]
            qkT_sb = big.tile([P, 8, QS], fp16, tag="qkT")
            zav_sb = big.tile([P, 16, 16], f8e4, tag="zav")

            for c in range(4):
                nc.sync.dma_start(xT_sb[:, ts(c, 2), :], xT[:, ts(c, 2), :])
            nc.sync.dma_start(w_sb[:], w[:])
            for c in range(8):
                nc.sync.dma_start(mask_sb[c][:], maskT16[:, ts(c, 2), :])

            nc.any.memset(zav_sb[:, :, 0:1], 1.0)
            biasC0 = big.tile([P, 1], mybir.dt.float32, tag="biasC0")
            nc.any.memset(biasC0[:], -C0)

            # one-time prologue: head-pair 0's projections (inside the loop,
            # hp3 projects hp0 for the NEXT iteration)
            DR = mybir.MatmulPerfMode.DoubleRow
            for m in (0, 4):
                for n4 in range(4):
                    ps0 = pz.tile([P, 512], mybir.dt.float32, tag="proj", bufs=1, name="ps0")
                    for c in range(4):
                        nc.tensor.matmul(
                            ps0,
                            lhsT=w_sb[:, 2 * c : 2 * c + 2, ts(m, P)],
                            rhs=xT_sb[:, 2 * c : 2 * c + 2, ts(n4, 512)],
                            start=(c == 0),
                            stop=(c == 3),
                            perf_mode=DR,
                        )
                    nc.vector.tensor_copy(qkT_sb[:, m, ts(n4, 512)], ps0)

            def body():
                _emit_body(nc, mybir, bass, pp, pz, work, xT_sb, w_sb, mask_sb, qkT_sb, zav_sb, biasC0, za)

            if loop_iters == 1:
                body()
            else:
                with tc.For_i(0, loop_iters, 1):
                    body()

    nc.compile()
    return nc


def _emit_body(nc, mybir, bass, pp, pz, work, xT_sb, w_sb, mask_sb, qkT_sb, zav_sb, biasC0, za):
    fp16 = mybir.dt.float16
    f32 = mybir.dt.float32
    f8e4 = mybir.dt.float8e4
    f8e5 = mybir.dt.float8e5
    i8 = mybir.dt.int8
    i16 = mybir.dt.int16
    ts = bass.ts
    ds = bass.ds
    Exp = mybir.ActivationFunctionType.Exp
    add = mybir.AluOpType.add
    band = mybir.AluOpType.bitwise_and
    DR = mybir.MatmulPerfMode.DoubleRow

    copy_flip = [0]

    def emit_proj_chunk(m, n4):
        # one 512-col PSUM tile of qkT group m, fp8e4 DoubleRow over
        # 4 x 256-hidden chunks
        ps = pz.tile([P, 512], f32, tag="proj", bufs=1, name="ps")
        for c in range(4):
            nc.tensor.matmul(
                ps,
                lhsT=w_sb[:, 2 * c : 2 * c + 2, ts(m, P)],
                rhs=xT_sb[:, 2 * c : 2 * c + 2, ts(n4, 512)],
                start=(c == 0),
                stop=(c == 3),
                perf_mode=DR,
            )
        nc.vector.tensor_copy(qkT_sb[:, m, ts(n4, 512)], ps)

    def emit_vproj(kt):
        # v projection for one k-tile -> zav[kt, 1:9] (fp8e4 for DR weights)
        psv = pz.tile([P, 512], f32, tag="proj", bufs=1, name="psv")
        for c in range(8):
            nc.tensor.matmul(
                psv[:, 0:8],
                lhsT=xT_sb[:, c, ts(kt, P)],
                rhs=w_sb[:, c, 1024:1032],
                start=(c == 0),
                stop=(c == 7),
            )
        nc.vector.tensor_copy(zav_sb[:, kt, 1:9], psv[:, 0:8])

    SQ = 1024 - SCH_Q  # exp region width within the [A|B] stacked psum

    for hp in range(4):
        hA = 2 * hp
        hB = 2 * hp + 1
        # proj chunks of the NEXT head-pair (hp3 projects hp0 for the next
        # loop iteration -- the one-time prologue outside the loop seeds it)
        nm = (hp + 1) % 4
        next_chunks = [(m, n4) for m in (nm, 4 + nm) for n4 in range(4)]
        for qw in range(4):  # 512-q windows
            zq = [
                pz.tile([2, 512], f32, tag=f"zq{j}", name=f"zq{j}") for j in range(2)
            ]
            # zq1 takes two matmuls per pair (region split) and start=True
            # clears has_written for the WHOLE bank -> seed it with a zero
            # matmul (w8 pad cols are zero) and always accumulate into it
            nc.tensor.matmul(
                zq[1][0:2, :],
                lhsT=w_sb[:, 0, 1032:1034],
                rhs=xT_sb[:, 0, 0:512],
                start=True,
                stop=False,
                skip_group_check=True,
            )

            def emit_za(k0, esc, last):
                selA = zav_sb[:, k0 : k0 + 2, 0 : (2 + hA) : (1 + hA)]
                selB = zav_sb[:, k0 : k0 + 2, 0 : (2 + hB) : (1 + hB)]
                bd = SQ - 512  # head-B e4m3/e5m2 region boundary
                for sel, rhs, out, st in (
                    (selA, esc[:, :, 0:512], zq[0][0:2, :], k0 == 0),
                    (selB, esc[:, :, 512:SQ], zq[1][0:2, 0:bd], False),
                    (selB, esc[:, :, SQ:1024].bitcast(f8e5), zq[1][0:2, bd:512], False),
                ):
                    nc.tensor.matmul(
                        out,
                        lhsT=sel,
                        rhs=rhs,
                        start=st,
                        stop=last,
                        perf_mode=DR,
                        skip_group_check=True,
                    )

            def emit_and(esc):
                # mask: one int16 AND over a pair's fp8 esc -- emitted one
                # pair LATE so it runs in DVE slack and never delays the
                # next TS (which gates the score-psum buffer release)
                v16 = esc[:].rearrange("p a (h q) -> p a h q", h=2).bitcast(i16)
                kk = and_pending[1]
                msl = mask_sb[kk // 2][:, :, ds(qw * 256, 256)]
                m16b = msl.unsqueeze(2).to_broadcast([P, 2, 2, 256])
                nc.vector.tensor_tensor(v16, v16, m16b, band)

            and_pending = None
            za_pending = []
            esc8 = None
            # single-bank Schraudolph score tile, parity ping-pong by slicing
            psT2 = pp.tile([P, 2, SCH_Q], f32, tag="scT", bufs=1, name="psT2")
            for kt in range(16):
                par = kt % 2
                if hp == 0 and qw == 0:
                    emit_vproj(kt)
                if kt % 4 == 1:
                    ci = qw * 4 + kt // 4
                    if ci % 2 == 0 and ci // 2 < len(next_chunks):
                        emit_proj_chunk(*next_chunks[ci // 2])
                if par == 0:
                    esc8 = work.tile([P, 2, 1024], f8e4, tag="esc", bufs=6, name="esc8")
                ps = pp.tile([P, SQ], f32, tag="sc", bufs=2, name="ps")
                psT = psT2[:, par, :]
                nc.tensor.matmul(
                    ps[:, 0:512],
                    lhsT=qkT_sb[0:64, 4 + hp, ts(kt, P)],
                    rhs=qkT_sb[0:64, hp, ds(qw * 512, 512)],
                    start=True,
                    stop=True,
                    tile_position=(0, 0),
                )
                nc.tensor.matmul(
                    ps[:, 512:SQ],
                    lhsT=qkT_sb[64:P, 4 + hp, ts(kt, P)],
                    rhs=qkT_sb[64:P, hp, ds(qw * 512, SQ - 512)],
                    start=True,
                    stop=True,
                    tile_position=(64, 0),
                )
                # Schraudolph region scores land in their own psum tile so
                # the main tile is released by the exp alone (a late TS can
                # no longer stall the score pipeline)
                nc.tensor.matmul(
                    psT,
                    lhsT=qkT_sb[64:P, 4 + hp, ts(kt, P)],
                    rhs=qkT_sb[64:P, hp, ds(qw * 512 + SQ - 512, SCH_Q)],
                    start=True,
                    stop=True,
                    tile_position=(64, 0),
                )
                # Z/A of an older pair behind this k-tile's scores
                if par == 0 and len(za_pending) > 0 and kt >= 4:
                    emit_za(*za_pending.pop(0), False)
                # head A + head B upper: ACT true exp; head B q-tail: DVE
                # Schraudolph (int8 RNE -> e5m2 bits)
                nc.scalar.activation(esc8[:, par, 0:SQ], ps[:], Exp, bias=biasC0[:], scale=1.0 / A8)
                nc.vector.tensor_scalar(
                    esc8[:, par, SQ:1024].bitcast(i8), psT[:], 0.5, SCH_B5,
                    mybir.AluOpType.mult, add,
                )
                if par == 1:
                    if and_pending is not None:
                        emit_and(and_pending[0])
                        za_pending.append((and_pending[1] - 1, and_pending[0]))
                    and_pending = (esc8, kt)
            emit_and(and_pending[0])
            za_pending.append((and_pending[1] - 1, and_pending[0]))
            for i, p in enumerate(za_pending):
                emit_za(*p, i == len(za_pending) - 1)
            stq = work.tile([2, 2, 512], f32, tag="stq", bufs=2, name="stq")
            nc.vector.tensor_copy(stq[:, 0, :], zq[0][0:2, :])
            nc.vector.tensor_copy(stq[:, 1, :], zq[1][0:2, :])
            for j, h in enumerate((hA, hB)):
                nc.sync.dma_start(
                    za[:, h, ds(qw * 512, 512)],
                    stq[:, j, :],
                )


def _get_nc():
    if "nc" not in _CACHE:
        _CACHE["nc"] = _build_bass()
    return _CACHE["nc"]


def _pack_128(a):
    """[R, F] row-major -> [128, R//128, F] with [p, c, f] = a[128c+p, f]."""
    r, f = a.shape
    return np.ascontiguousarray(a.reshape(r // P, P, f).transpose(1, 0, 2))


def _to_e4(a):
    import jax.numpy as jnp

    return np.asarray(jnp.asarray(a, dtype=jnp.float8_e4m3))


def make_in_maps(x, att_mask, W_qk, W_v):
    Wq = np.asarray(W_qk[:, : N_HEADS * HEAD_DIM]) * (A8 / np.sqrt(HEAD_DIM) * XSCALE)
    Wk = np.asarray(W_qk[:, N_HEADS * HEAD_DIM :]) * XSCALE
    Wv = np.asarray(W_v) * XSCALE
    in_maps = []
    for c in range(NCORES):
        b, g = divmod(c, 2)
        if g == 0:
            xT_b = _pack_128(_to_e4(np.asarray(x[b]).T / XSCALE))
            liveT = (~np.asarray(att_mask[b])).T  # [k, q]
            m16 = (
                liveT[:, 0::2] * 0x00FF + liveT[:, 1::2] * 0xFF00
            ).astype(np.uint16).view(np.int16)
            maskT16_b = _pack_128(m16)
        wc = np.concatenate(
            [
                Wq[:, 512 * g : 512 * (g + 1)],
                Wk[:, 512 * g : 512 * (g + 1)],
                Wv[:, HPC * g : HPC * (g + 1)],
                np.zeros((HID, 8), np.float32),
            ],
            axis=1,
        )
        in_maps.append({"xT": xT_b, "maskT16": maskT16_b, "w": _pack_128(_to_e4(wc))})
    return in_maps


def _combine(za_list, att_mask):
    bs = att_mask.shape[0]
    attended = np.zeros((bs, QS), np.float64)
    for c in range(NCORES):
        b = c // 2
        z = za_list[c][0].astype(np.float64)  # [8, QS]
        a = za_list[c][1].astype(np.float64)
        attended[b] += (a / z).sum(axis=0)
    pm = np.asarray(att_mask[:, -1])
    o = np.where(pm, NEG, attended)
    out = np.where(o >= 0, 1.0 / (1.0 + np.exp(-np.clip(o, 0, None))),
                   np.exp(np.clip(o, None, 0)) / (1.0 + np.exp(np.clip(o, None, 0))))
    return out[..., None].astype(np.float32)


def kernel(x, att_mask, W_qk, W_v):
    from concourse.bass_utils import run_bass_kernel_spmd

    nc = _get_nc()
    in_maps = make_in_maps(x, att_mask, W_qk, W_v)
    res = run_bass_kernel_spmd(nc, in_maps, core_ids=list(range(NCORES)))
    _CACHE["last_results"] = res
    za_list = [r["za"] for r in res.results]
    return _combine(za_list, np.asarray(att_mask))


def _make_runner(nc):
    """Cached-jit SPMD runner modeled on bass2jax.run_bass_via_pjrt (no
    donation so device-resident inputs survive across calls)."""
    import jax
    from jax.sharding import Mesh, PartitionSpec
    from jax.experimental.shard_map import shard_map

    import concourse.mybir as mybir
    from concourse import bass2jax

    bass2jax.install_neuronx_cc_hook()
    partition_name = nc.partition_id_tensor.name if nc.partition_id_tensor else None
    in_names, out_names, out_avals, zero_outs = [], [], [], []
    for alloc in nc.m.functions[0].allocations:
        if not isinstance(alloc, mybir.MemoryLocationSet):
            continue
        name = alloc.memorylocations[0].name
        if alloc.kind == "ExternalInput":
            if name != partition_name:
                in_names.append(name)
        elif alloc.kind == "ExternalOutput":
            shape = tuple(alloc.tensor_shape)
            dtype = mybir.dt.np(alloc.dtype)
            out_names.append(name)
            out_avals.append(jax.core.ShapedArray(shape, dtype))
            zero_outs.append(np.zeros(shape, dtype))
    n_params = len(in_names)
    all_in_names = in_names + out_names
    if partition_name is not None:
        all_in_names.append(partition_name)

    def _body(*args):
        operands = list(args)
        if partition_name is not None:
            operands.append(bass2jax.partition_id_tensor())
        outs = bass2jax._bass_exec_p.bind(
            *operands,
            out_avals=tuple(out_avals),
            in_names=tuple(all_in_names),
            out_names=tuple(out_names),
            lowering_input_output_aliases=(),
            sim_require_finite=True,
            sim_require_nnan=True,
            nc=nc,
        )
        return tuple(outs)

    devices = jax.devices()[:NCORES]
    mesh = Mesh(np.asarray(devices), ("core",))
    in_specs = (PartitionSpec("core"),) * (n_params + len(out_names))
    out_specs = (PartitionSpec("core"),) * len(out_names)
    sharded = jax.jit(
        shard_map(_body, mesh=mesh, in_specs=in_specs, out_specs=out_specs, check_rep=False),
        keep_unused=True,
    )

    def put(in_maps):
        concat_in = [
            np.concatenate([np.asarray(in_maps[c][nm]) for c in range(NCORES)], axis=0)
            for nm in in_names
        ]
        concat_zero = [np.zeros((NCORES * z.shape[0], *z.shape[1:]), z.dtype) for z in zero_outs]
        return [jax.device_put(a) for a in concat_in + concat_zero]

    def run(dev_args):
        outs = sharded(*dev_args)
        jax.block_until_ready(outs)
        return outs

    def unpack(outs):
        return [
            {nm: np.asarray(outs[i]).reshape(NCORES, *out_avals[i].shape)[c]
             for i, nm in enumerate(out_names)}
            for c in range(NCORES)
        ]

    return put, run, unpack


def bench(x, att_mask, W_qk, W_v, k=1025, reps=4):
    """Estimate per-iteration device time via For_i loop-count delta."""
    import time

    in_maps = make_in_maps(x, att_mask, W_qk, W_v)
    walls = {}
    for iters in (1, k):
        nc = _build_bass(loop_iters=iters)
        put, run, unpack = _make_runner(nc)
        dev_args = put(in_maps)
        run(dev_args)  # warm (compile)
        ts = []
        for _ in range(reps):
            t0 = time.monotonic()
            run(dev_args)
            ts.append(time.monotonic() - t0)
        walls[iters] = ts
        print(f"iters={iters}: walls {' '.join(f'{t*1e3:.1f}ms' for t in ts)}")
    per_iter = (min(walls[k]) - min(walls[1])) / (k - 1)
    print(f"per-iteration device time: {per_iter*1e6:.1f} us")
    print(f"HW exec time: {per_iter*1e9:.0f} ns")
    return per_iter


# revision 26
# speedup vs baseline: 1.2873x; 1.2873x over previous
"""Trainium2 Bass kernel for nn_BoundaryDecision (sparse attention with scalar V).

Math: out = sigmoid(mask_last_row(  sum_n softmax_k(mask(q_n . k_n / sqrt(d)))  @ v_n ))
Key identity used: per-head V dim is 1, so we never materialize prob:
    attended_n[q] = A_n[q] / Z_n[q]
    Z_n[q] = sum_k maskc[q,k] * exp(s_n[q,k])
    A_n[q] = sum_k maskc[q,k] * exp(s_n[q,k]) * v_n[k]
Both are PE contractions over k of the masked escore^T tensor.

Sharding (8 cores): core c -> batch b=c//2, head-group g=c%2 (8 heads each).
Each core returns Z,A per (head, q); host does A/Z, head-sum across the two
head-group cores, final padded-mask + sigmoid.

v5 design (engine-balanced against the TimelineSim cost model, which matches
HW within ~2%; PE matmuls cost out_cols cycles SERIALLY -- tile_position
concurrency is NOT real on this target):
  - exp work split per k-tile: head A (1024 q) + head B q[512:1024] on ACT
    (true exp, fp8e5 out); head B q[0:512] on DVE as Schraudolph: scores
    pre-scaled by A8=4/ln2 (folded into Wq), tensor_scalar adds B8=59.8 and
    the int8 RNE convert yields e5m2 bits ~= exp(s) (HW-verified).
  - mask: one int16 bitwise-AND per k-tile over the fp8 esc pair (2x DVE
    mode; host packs {0x0000,0x00FF,0xFF00,0xFFFF} per q-pair).
  - Z/A: fp8 DoubleRow matmuls, contraction 256 = two k-tiles per stream
    (0.5 cyc/out-col), esc pair tiles [P, 2(kt parity), 2(head), 1024].
    DR outputs must sit at PSUM partition 0 -> four single-bank
    accumulators; evacuated by DMA straight from PSUM to DRAM.
  - proj: fp8e4 DoubleRow (x/16 and W*16 to stay in e4m3 normal range),
    4 x 256-contraction matmuls per 512-col tile, interleaved into the
    previous head-pair's k-loop.  Score matmuls stay fp16.
  - Z/A matmuls of a pair go behind the next scores in the PE FIFO so they
    never gate them (strict in-order engines).
"""

import os

import numpy as np

NEG = -60000.0
P = 128
QS = 2048
HID = 1024
N_HEADS = 16
HEAD_DIM = 64
NCORES = 8
HPC = 8  # heads per core

A8 = 8.0 / float(np.log(2.0))  # 11.5416 (Schraudolph e4m3 scale)
C0 = 4.0  # ACT-region exp shift: esc stores exp(s-4) in e4m3 (ceiling s=9.5)
C0B = 2.0  # Schraudolph-region shift (e5m2, ceiling s=13)
A5 = 4.0 / float(np.log(2.0))  # e5m2 Schraudolph scale
SCH_B5 = 59.8 - 2.0 * 4.0 / float(np.log(2.0))  # e5m2 offset w/ C0B shift
SCH_Q = 304  # q-columns per 512-q window handled by the DVE Schraudolph path
XSCALE = 16.0  # x/16, W*16 keeps fp8e4 operands in the normal range

_CACHE = {}


def _build_bass(loop_iters=1):
    import concourse.bass as bass
    import concourse.mybir as mybir
    from concourse import bacc, tile

    fp16 = mybir.dt.float16
    f32 = mybir.dt.float32
    f8e4 = mybir.dt.float8e4
    f8e5 = mybir.dt.float8e5
    i16 = mybir.dt.int16
    ts = bass.ts

    nc = bacc.Bacc(trn_type="TRN2")

    xT = nc.declare_dram_parameter("xT", [P, 8, QS], f8e4, isOutput=False)
    w = nc.declare_dram_parameter("w", [P, 8, 1040], f8e4, isOutput=False)
    maskT16 = nc.declare_dram_parameter("maskT16", [P, 16, QS // 2], i16, isOutput=False)
    za = nc.declare_dram_parameter("za", [2, HPC, QS], f32, isOutput=True)

    with tile.TileContext(nc) as tc:
        with (
            tc.tile_pool(name="big", bufs=1) as big,
            tc.tile_pool(name="work", bufs=3) as work,
            tc.tile_pool(name="psum", bufs=1, space="PSUM") as pp,
            tc.tile_pool(name="psum_za", bufs=1, space="PSUM") as pz,
        ):
            xT_sb = big.tile([P, 8, QS], f8e4, tag="xT")
            w_sb = big.tile([P, 8, 1040], f8e4, tag="w")
            # one tile per 2-k-tile chunk so the first mask consumer only
            # waits on its own chunk's DMA
            mask_sb = [
                big.tile([P, 2, QS // 2], i16, tag=f"mask{c}", name=f"mask{c}")
                for c in range(8)
            ]
            qkT_sb = big.tile([P, 8, QS], fp16, tag="qkT")
            zav_sb = big.tile([P, 16, 16], f8e4, tag="zav")

            for c in range(4):
                nc.sync.dma_start(xT_sb[:, ts(c, 2), :], xT[:, ts(c, 2), :])
            nc.sync.dma_start(w_sb[:], w[:])
            for c in range(8):
                nc.sync.dma_start(mask_sb[c][:], maskT16[:, ts(c, 2), :])

            nc.any.memset(zav_sb[:, :, 0:1], 1.0)
            biasC0 = big.tile([P, 1], mybir.dt.float32, tag="biasC0")
            nc.any.memset(biasC0[:], -C0)

            # one-time prologue: head-pair 0's projections (inside the loop,
            # hp3 projects hp0 for the NEXT iteration)
            DR = mybir.MatmulPerfMode.DoubleRow
            for m in (0, 4):
                for n4 in range(4):
                    ps0 = pz.tile([P, 512], mybir.dt.float32, tag="proj", bufs=1, name="ps0")
                    for c in range(4):
                        nc.tensor.matmul(
                            ps0,
                            lhsT=w_sb[:, 2 * c : 2 * c + 2, ts(m, P)],
                            rhs=xT_sb[:, 2 * c : 2 * c + 2, ts(n4, 512)],
                            start=(c == 0),
                            stop=(c == 3),
                            perf_mode=DR,
                        )
                    nc.vector.tensor_copy(qkT_sb[:, m, ts(n4, 512)], ps0)

            def body():
                _emit_body(nc, mybir, bass, pp, pz, work, xT_sb, w_sb, mask_sb, qkT_sb, zav_sb, biasC0, za)

            if loop_iters == 1:
                body()
            else:
                with tc.For_i(0, loop_iters, 1):
                    body()

    nc.compile()
    return nc


def _emit_body(nc, mybir, bass, pp, pz, work, xT_sb, w_sb, mask_sb, qkT_sb, zav_sb, biasC0, za):
    fp16 = mybir.dt.float16
    f32 = mybir.dt.float32
    f8e4 = mybir.dt.float8e4
    f8e5 = mybir.dt.float8e5
    i8 = mybir.dt.int8
    i16 = mybir.dt.int16
    ts = bass.ts
    ds = bass.ds
    Exp = mybir.ActivationFunctionType.Exp
    add = mybir.AluOpType.add
    band = mybir.AluOpType.bitwise_and
    DR = mybir.MatmulPerfMode.DoubleRow

    copy_flip = [0]

    def emit_proj_chunk(m, n4):
        # one 512-col PSUM tile of qkT group m, fp8e4 DoubleRow over
        # 4 x 256-hidden chunks
        ps = pz.tile([P, 512], f32, tag="proj", bufs=1, name="ps")
        for c in range(4):
            nc.tensor.matmul(
                ps,
                lhsT=w_sb[:, 2 * c : 2 * c + 2, ts(m, P)],
                rhs=xT_sb[:, 2 * c : 2 * c + 2, ts(n4, 512)],
                start=(c == 0),
                stop=(c == 3),
                perf_mode=DR,
            )
        dst = qkT_sb[:, m, ts(n4, 512)]
        if copy_flip[0] % 2 == 0:
            nc.vector.tensor_copy(dst, ps)
        else:
            nc.scalar.copy(dst, ps)
        copy_flip[0] += 1

    def emit_vproj(kt):
        # v projection for one k-tile -> zav[kt, 1:9] (fp8e4 for DR weights)
        psv = pz.tile([P, 512], f32, tag="proj", bufs=1, name="psv")
        for c in range(8):
            nc.tensor.matmul(
                psv[:, 0:8],
                lhsT=xT_sb[:, c, ts(kt, P)],
                rhs=w_sb[:, c, 1024:1032],
                start=(c == 0),
                stop=(c == 7),
            )
        nc.vector.tensor_copy(zav_sb[:, kt, 1:9], psv[:, 0:8])

    SQ = 1024 - SCH_Q  # exp region width within the [A|B] stacked psum

    for hp in range(4):
        hA = 2 * hp
        hB = 2 * hp + 1
        # proj chunks of the NEXT head-pair (hp3 projects hp0 for the next
        # loop iteration -- the one-time prologue outside the loop seeds it)
        nm = (hp + 1) % 4
        next_chunks = [(m, n4) for m in (nm, 4 + nm) for n4 in range(4)]
        for qw in range(4):  # 512-q windows
            zq = [
                pz.tile([2, 512], f32, tag=f"zq{j}", name=f"zq{j}") for j in range(2)
            ]
            # zq1 takes two matmuls per pair (region split) and start=True
            # clears has_written for the WHOLE bank -> seed it with a zero
            # matmul (w8 pad cols are zero) and always accumulate into it
            nc.tensor.matmul(
                zq[1][0:2, :],
                lhsT=w_sb[:, 0, 1032:1034],
                rhs=xT_sb[:, 0, 0:512],
                start=True,
                stop=False,
                skip_group_check=True,
            )

            def emit_za(k0, esc, last):
                selA = zav_sb[:, k0 : k0 + 2, 0 : (2 + hA) : (1 + hA)]
                selB = zav_sb[:, k0 : k0 + 2, 0 : (2 + hB) : (1 + hB)]
                bd = SQ - 512  # head-B e4m3/e5m2 region boundary
                for sel, rhs, out, st in (
                    (selA, esc[:, :, 0:512], zq[0][0:2, :], k0 == 0),
                    (selB, esc[:, :, 512:SQ], zq[1][0:2, 0:bd], False),
                    (selB, esc[:, :, SQ:1024].bitcast(f8e5), zq[1][0:2, bd:512], False),
                ):
                    nc.tensor.matmul(
                        out,
                        lhsT=sel,
                        rhs=rhs,
                        start=st,
                        stop=last,
                        perf_mode=DR,
                        skip_group_check=True,
                    )

            def emit_and(esc):
                # mask: one int16 AND over a pair's fp8 esc
                kk = and_pending[1]
                v16 = esc[:].rearrange("p a (h q) -> p a h q", h=2).bitcast(i16)
                msl = mask_sb[kk // 2][:, :, ds(qw * 256, 256)]
                m16b = msl.unsqueeze(2).to_broadcast([P, 2, 2, 256])
                nc.vector.tensor_tensor(v16, v16, m16b, band)

            and_pending = None
            za_pending = []
            esc8 = None
            for kt in range(16):
                par = kt % 2
                if hp == 0 and qw == 0:
                    emit_vproj(kt)
                if kt % 4 == 1:
                    ci = qw * 4 + kt // 4
                    if ci % 2 == 0 and ci // 2 < len(next_chunks):
                        emit_proj_chunk(*next_chunks[ci // 2])
                if par == 0:
                    esc8 = work.tile([P, 2, 1024], f8e4, tag="esc", bufs=6, name="esc8")
                ps = pp.tile([P, 1024], f32, tag="sc", bufs=2, name="ps")
                nc.tensor.matmul(
                    ps[:, 0:512],
                    lhsT=qkT_sb[0:64, 4 + hp, ts(kt, P)],
                    rhs=qkT_sb[0:64, hp, ds(qw * 512, 512)],
                    start=True,
                    stop=True,
                    tile_position=(0, 0),
                )
                nc.tensor.matmul(
                    ps[:, 512:1024],
                    lhsT=qkT_sb[64:P, 4 + hp, ts(kt, P)],
                    rhs=qkT_sb[64:P, hp, ds(qw * 512, 512)],
                    start=True,
                    stop=True,
                    tile_position=(64, 0),
                )
                # Z/A of an older pair behind this k-tile's scores
                if par == 0 and len(za_pending) > 0 and kt >= 4:
                    emit_za(*za_pending.pop(0), False)
                # head A + head B upper: ACT true exp; head B q-tail: DVE
                # Schraudolph (int8 RNE -> e5m2 bits)
                nc.scalar.activation(esc8[:, par, 0:SQ], ps[:, 0:SQ], Exp, bias=biasC0[:], scale=1.0 / A8)
                nc.vector.tensor_scalar(
                    esc8[:, par, SQ:1024].bitcast(i8), ps[:, SQ:1024], 0.5, SCH_B5,
                    mybir.AluOpType.mult, add,
                )
                if par == 1:
                    if and_pending is not None:
                        emit_and(and_pending[0])
                        za_pending.append((and_pending[1] - 1, and_pending[0]))
                    and_pending = (esc8, kt)
            emit_and(and_pending[0])
            za_pending.append((and_pending[1] - 1, and_pending[0]))
            for i, p in enumerate(za_pending):
                emit_za(*p, i == len(za_pending) - 1)
            stq = work.tile([2, 2, 512], f32, tag="stq", bufs=2, name="stq")
            nc.vector.tensor_copy(stq[:, 0, :], zq[0][0:2, :])
            nc.scalar.copy(stq[:, 1, :], zq[1][0:2, :])
            for j, h in enumerate((hA, hB)):
                nc.sync.dma_start(
                    za[:, h, ds(qw * 512, 512)],
                    stq[:, j, :],
                )


def _get_nc():
    if "nc" not in _CACHE:
        _CACHE["nc"] = _build_bass()
    return _CACHE["nc"]


def _pack_128(a):
    """[R, F] row-major -> [128, R//128, F] with [p, c, f] = a[128c+p, f]."""
    r, f = a.shape
    return np.ascontiguousarray(a.reshape(r // P, P, f).transpose(1, 0, 2))


def _to_e4(a):
    import jax.numpy as jnp

    return np.asarray(jnp.asarray(a, dtype=jnp.float8_e4m3))


def make_in_maps(x, att_mask, W_qk, W_v):
    Wq = np.asarray(W_qk[:, : N_HEADS * HEAD_DIM]) * (A8 / np.sqrt(HEAD_DIM) * XSCALE)
    Wk = np.asarray(W_qk[:, N_HEADS * HEAD_DIM :]) * XSCALE
    Wv = np.asarray(W_v) * XSCALE
    in_maps = []
    for c in range(NCORES):
        b, g = divmod(c, 2)
        if g == 0:
            xT_b = _pack_128(_to_e4(np.asarray(x[b]).T / XSCALE))
            liveT = (~np.asarray(att_mask[b])).T  # [k, q]
            m16 = (
                liveT[:, 0::2] * 0x00FF + liveT[:, 1::2] * 0xFF00
            ).astype(np.uint16).view(np.int16)
            maskT16_b = _pack_128(m16)
        wc = np.concatenate(
            [
                Wq[:, 512 * g : 512 * (g + 1)],
                Wk[:, 512 * g : 512 * (g + 1)],
                Wv[:, HPC * g : HPC * (g + 1)],
                np.zeros((HID, 8), np.float32),
            ],
            axis=1,
        )
        in_maps.append({"xT": xT_b, "maskT16": maskT16_b, "w": _pack_128(_to_e4(wc))})
    return in_maps


def _combine(za_list, att_mask):
    bs = att_mask.shape[0]
    attended = np.zeros((bs, QS), np.float64)
    for c in range(NCORES):
        b = c // 2
        z = za_list[c][0].astype(np.float64)  # [8, QS]
        a = za_list[c][1].astype(np.float64)
        attended[b] += (a / z).sum(axis=0)
    pm = np.asarray(att_mask[:, -1])
    o = np.where(pm, NEG, attended)
    out = np.where(o >= 0, 1.0 / (1.0 + np.exp(-np.clip(o, 0, None))),
                   np.exp(np.clip(o, None, 0)) / (1.0 + np.exp(np.clip(o, None, 0))))
    return out[..., None].astype(np.float32)


def kernel(x, att_mask, W_qk, W_v):
    from concourse.bass_utils import run_bass_kernel_spmd

    nc = _get_nc()
    in_maps = make_in_maps(x, att_mask, W_qk, W_v)
    res = run_bass_kernel_spmd(nc, in_maps, core_ids=list(range(NCORES)))
    _CACHE["last_results"] = res
    za_list = [r["za"] for r in res.results]
    return _combine(za_list, np.asarray(att_mask))


def _make_runner(nc):
    """Cached-jit SPMD runner modeled on bass2jax.run_bass_via_pjrt (no
    donation so device-resident inputs survive across calls)."""
    import jax
    from jax.sharding import Mesh, PartitionSpec
    from jax.experimental.shard_map import shard_map

    import concourse.mybir as mybir
    from concourse import bass2jax

    bass2jax.install_neuronx_cc_hook()
    partition_name = nc.partition_id_tensor.name if nc.partition_id_tensor else None
    in_names, out_names, out_avals, zero_outs = [], [], [], []
    for alloc in nc.m.functions[0].allocations:
        if not isinstance(alloc, mybir.MemoryLocationSet):
            continue
        name = alloc.memorylocations[0].name
        if alloc.kind == "ExternalInput":
            if name != partition_name:
                in_names.append(name)
        elif alloc.kind == "ExternalOutput":
            shape = tuple(alloc.tensor_shape)
            dtype = mybir.dt.np(alloc.dtype)
            out_names.append(name)
            out_avals.append(jax.core.ShapedArray(shape, dtype))
            zero_outs.append(np.zeros(shape, dtype))
    n_params = len(in_names)
    all_in_names = in_names + out_names
    if partition_name is not None:
        all_in_names.append(partition_name)

    def _body(*args):
        operands = list(args)
        if partition_name is not None:
            operands.append(bass2jax.partition_id_tensor())
        outs = bass2jax._bass_exec_p.bind(
            *operands,
            out_avals=tuple(out_avals),
            in_names=tuple(all_in_names),
            out_names=tuple(out_names),
            lowering_input_output_aliases=(),
            sim_require_finite=True,
            sim_require_nnan=True,
            nc=nc,
        )
        return tuple(outs)

    devices = jax.devices()[:NCORES]
    mesh = Mesh(np.asarray(devices), ("core",))
    in_specs = (PartitionSpec("core"),) * (n_params + len(out_names))
    out_specs = (PartitionSpec("core"),) * len(out_names)
    sharded = jax.jit(
        shard_map(_body, mesh=mesh, in_specs=in_specs, out_specs=out_specs, check_rep=False),
        keep_unused=True,
    )

    def put(in_maps):
        concat_in = [
            np.concatenate([np.asarray(in_maps[c][nm]) for c in range(NCORES)], axis=0)
            for nm in in_names
        ]
        concat_zero = [np.zeros((NCORES * z.shape[0], *z.shape[1:]), z.dtype) for z in zero_outs]
        return [jax.device_put(a) for a in concat_in + concat_zero]

    def run(dev_args):
        outs = sharded(*dev_args)
        jax.block_until_ready(outs)
        return outs

    def unpack(outs):
        return [
            {nm: np.asarray(outs[i]).reshape(NCORES, *out_avals[i].shape)[c]
             for i, nm in enumerate(out_names)}
            for c in range(NCORES)
        ]

    return put, run, unpack


def bench(x, att_mask, W_qk, W_v, k=1025, reps=4):
    """Estimate per-iteration device time via For_i loop-count delta."""
    import time

    in_maps = make_in_maps(x, att_mask, W_qk, W_v)
    walls = {}
    for iters in (1, k):
        nc = _build_bass(loop_iters=iters)
        put, run, unpack = _make_runner(nc)
        dev_args = put(in_maps)
        run(dev_args)  # warm (compile)
        ts = []
        for _ in range(reps):
            t0 = time.monotonic()
            run(dev_args)
            ts.append(time.monotonic() - t0)
        walls[iters] = ts
        print(f"iters={iters}: walls {' '.join(f'{t*1e3:.1f}ms' for t in ts)}")
    per_iter = (min(walls[k]) - min(walls[1])) / (k - 1)
    print(f"per-iteration device time: {per_iter*1e6:.1f} us")
    print(f"HW exec time: {per_iter*1e9:.0f} ns")
    return per_iter


# revision 27
# speedup vs baseline: 1.4481x; 1.1249x over previous
"""Trainium2 Bass kernel for nn_BoundaryDecision (sparse attention with scalar V).

Math: out = sigmoid(mask_last_row(  sum_n softmax_k(mask(q_n . k_n / sqrt(d)))  @ v_n ))
Key identity used: per-head V dim is 1, so we never materialize prob:
    attended_n[q] = A_n[q] / Z_n[q]
    Z_n[q] = sum_k maskc[q,k] * exp(s_n[q,k])
    A_n[q] = sum_k maskc[q,k] * exp(s_n[q,k]) * v_n[k]
Both are PE contractions over k of the masked escore^T tensor.

Sharding (8 cores): core c -> batch b=c//2, head-group g=c%2 (8 heads each).
Each core returns Z,A per (head, q); host does A/Z, head-sum across the two
head-group cores, final padded-mask + sigmoid.

v5 design (engine-balanced against the TimelineSim cost model, which matches
HW within ~2%; PE matmuls cost out_cols cycles SERIALLY -- tile_position
concurrency is NOT real on this target):
  - exp work split per k-tile: head A (1024 q) + head B q[512:1024] on ACT
    (true exp, fp8e5 out); head B q[0:512] on DVE as Schraudolph: scores
    pre-scaled by A8=4/ln2 (folded into Wq), tensor_scalar adds B8=59.8 and
    the int8 RNE convert yields e5m2 bits ~= exp(s) (HW-verified).
  - mask: one int16 bitwise-AND per k-tile over the fp8 esc pair (2x DVE
    mode; host packs {0x0000,0x00FF,0xFF00,0xFFFF} per q-pair).
  - Z/A: fp8 DoubleRow matmuls, contraction 256 = two k-tiles per stream
    (0.5 cyc/out-col), esc pair tiles [P, 2(kt parity), 2(head), 1024].
    DR outputs must sit at PSUM partition 0 -> four single-bank
    accumulators; evacuated by DMA straight from PSUM to DRAM.
  - proj: fp8e4 DoubleRow (x/16 and W*16 to stay in e4m3 normal range),
    4 x 256-contraction matmuls per 512-col tile, interleaved into the
    previous head-pair's k-loop.  Score matmuls stay fp16.
  - Z/A matmuls of a pair go behind the next scores in the PE FIFO so they
    never gate them (strict in-order engines).
"""

import os

import numpy as np

NEG = -60000.0
P = 128
QS = 2048
HID = 1024
N_HEADS = 16
HEAD_DIM = 64
NCORES = 8
HPC = 8  # heads per core

A8 = 8.0 / float(np.log(2.0))  # 11.5416 (Schraudolph e4m3 scale)
C0 = 4.0  # ACT-region exp shift: esc stores exp(s-4) in e4m3 (ceiling s=9.5)
C0B = 2.0  # Schraudolph-region shift (e5m2, ceiling s=13)
A5 = 4.0 / float(np.log(2.0))  # e5m2 Schraudolph scale
SCH_B5 = 59.8 - 2.0 * 4.0 / float(np.log(2.0))  # e5m2 offset w/ C0B shift
SCH_Q = 304  # q-columns per 512-q window handled by the DVE Schraudolph path
XSCALE = 16.0  # x/16, W*16 keeps fp8e4 operands in the normal range

_CACHE = {}


def _build_bass(loop_iters=1):
    import concourse.bass as bass
    import concourse.mybir as mybir
    from concourse import bacc, tile

    fp16 = mybir.dt.float16
    f32 = mybir.dt.float32
    f8e4 = mybir.dt.float8e4
    f8e5 = mybir.dt.float8e5
    i16 = mybir.dt.int16
    ts = bass.ts

    nc = bacc.Bacc(trn_type="TRN2")

    xT = nc.declare_dram_parameter("xT", [P, 8, QS], f8e4, isOutput=False)
    w = nc.declare_dram_parameter("w", [P, 8, 1040], f8e4, isOutput=False)
    maskT16 = nc.declare_dram_parameter("maskT16", [P, 16, QS // 2], i16, isOutput=False)
    za = nc.declare_dram_parameter("za", [2, HPC, QS], f32, isOutput=True)

    with tile.TileContext(nc) as tc:
        with (
            tc.tile_pool(name="big", bufs=1) as big,
            tc.tile_pool(name="work", bufs=3) as work,
            tc.tile_pool(name="psum", bufs=1, space="PSUM") as pp,
            tc.tile_pool(name="psum_za", bufs=1, space="PSUM") as pz,
        ):
            xT_sb = big.tile([P, 8, QS], f8e4, tag="xT")
            w_sb = big.tile([P, 8, 1040], f8e4, tag="w")
            # one tile per 2-k-tile chunk so the first mask consumer only
            # waits on its own chunk's DMA
            mask_sb = [
                big.tile([P, 2, QS // 2], i16, tag=f"mask{c}", name=f"mask{c}")
                for c in range(8)
            ]
            qkT_sb = big.tile([P, 8, QS], fp16, tag="qkT")
            zav_sb = big.tile([P, 16, 16], f8e4, tag="zav")

            for c in range(4):
                nc.sync.dma_start(xT_sb[:, ts(c, 2), :], xT[:, ts(c, 2), :])
            nc.sync.dma_start(w_sb[:], w[:])
            for c in range(8):
                nc.sync.dma_start(mask_sb[c][:], maskT16[:, ts(c, 2), :])

            nc.any.memset(zav_sb[:, :, 0:1], 1.0)
            biasC0 = big.tile([P, 1], mybir.dt.float32, tag="biasC0")
            nc.any.memset(biasC0[:], -C0)

            # one-time prologue: head-pair 0's projections (inside the loop,
            # hp3 projects hp0 for the NEXT iteration)
            DR = mybir.MatmulPerfMode.DoubleRow
            for m in (0, 4):
                for n4 in range(4):
                    ps0 = pz.tile([P, 512], mybir.dt.float32, tag="proj", bufs=1, name="ps0")
                    for c in range(4):
                        nc.tensor.matmul(
                            ps0,
                            lhsT=w_sb[:, 2 * c : 2 * c + 2, ts(m, P)],
                            rhs=xT_sb[:, 2 * c : 2 * c + 2, ts(n4, 512)],
                            start=(c == 0),
                            stop=(c == 3),
                            perf_mode=DR,
                        )
                    nc.vector.tensor_copy(qkT_sb[:, m, ts(n4, 512)], ps0)

            def body():
                _emit_body(nc, mybir, bass, pp, pz, work, xT_sb, w_sb, mask_sb, qkT_sb, zav_sb, biasC0, za)

            if loop_iters == 1:
                body()
            else:
                with tc.For_i(0, loop_iters, 1):
                    body()

    nc.compile()
    return nc


def _emit_body(nc, mybir, bass, pp, pz, work, xT_sb, w_sb, mask_sb, qkT_sb, zav_sb, biasC0, za):
    fp16 = mybir.dt.float16
    f32 = mybir.dt.float32
    f8e4 = mybir.dt.float8e4
    f8e5 = mybir.dt.float8e5
    i8 = mybir.dt.int8
    i16 = mybir.dt.int16
    ts = bass.ts
    ds = bass.ds
    Exp = mybir.ActivationFunctionType.Exp
    add = mybir.AluOpType.add
    band = mybir.AluOpType.bitwise_and
    DR = mybir.MatmulPerfMode.DoubleRow

    copy_flip = [0]

    def emit_proj_chunk(m, n4):
        # one 512-col PSUM tile of qkT group m, fp8e4 DoubleRow over
        # 4 x 256-hidden chunks
        ps = pz.tile([P, 512], f32, tag="proj", bufs=1, name="ps")
        for c in range(4):
            nc.tensor.matmul(
                ps,
                lhsT=w_sb[:, 2 * c : 2 * c + 2, ts(m, P)],
                rhs=xT_sb[:, 2 * c : 2 * c + 2, ts(n4, 512)],
                start=(c == 0),
                stop=(c == 3),
                perf_mode=DR,
            )
        dst = qkT_sb[:, m, ts(n4, 512)]
        if copy_flip[0] % 2 == 0:
            nc.vector.tensor_copy(dst, ps)
        else:
            nc.scalar.copy(dst, ps)
        copy_flip[0] += 1

    def emit_vproj(kt):
        # v projection for one k-tile -> zav[kt, 1:9] (fp8e4 for DR weights)
        psv = pz.tile([P, 512], f32, tag="proj", bufs=1, name="psv")
        for c in range(8):
            nc.tensor.matmul(
                psv[:, 0:8],
                lhsT=xT_sb[:, c, ts(kt, P)],
                rhs=w_sb[:, c, 1024:1032],
                start=(c == 0),
                stop=(c == 7),
            )
        nc.vector.tensor_copy(zav_sb[:, kt, 1:9], psv[:, 0:8])

    SQ = 1024 - SCH_Q  # exp region width within the [A|B] stacked psum

    for hp in range(4):
        hA = 2 * hp
        hB = 2 * hp + 1
        # proj chunks of the NEXT head-pair (hp3 projects hp0 for the next
        # loop iteration -- the one-time prologue outside the loop seeds it)
        nm = (hp + 1) % 4
        next_chunks = [(m, n4) for m in (nm, 4 + nm) for n4 in range(4)]
        for qw in range(4):  # 512-q windows
            zq = [
                pz.tile([2, 512], f32, tag=f"zq{j}", name=f"zq{j}") for j in range(2)
            ]
            # zq1 takes two matmuls per pair (region split) and start=True
            # clears has_written for the WHOLE bank -> seed it with a zero
            # matmul (w8 pad cols are zero) and always accumulate into it
            nc.tensor.matmul(
                zq[1][0:2, :],
                lhsT=w_sb[:, 0, 1032:1034],
                rhs=xT_sb[:, 0, 0:512],
                start=True,
                stop=False,
                skip_group_check=True,
            )

            def emit_za(k0, esc, last):
                selA = zav_sb[:, k0 : k0 + 2, 0 : (2 + hA) : (1 + hA)]
                selB = zav_sb[:, k0 : k0 + 2, 0 : (2 + hB) : (1 + hB)]
                bd = SQ - 512  # head-B e4m3/e5m2 region boundary
                for sel, rhs, out, st in (
                    (selA, esc[:, :, 0:512], zq[0][0:2, :], k0 == 0),
                    (selB, esc[:, :, 512:SQ], zq[1][0:2, 0:bd], False),
                    (selB, esc[:, :, SQ:1024].bitcast(f8e5), zq[1][0:2, bd:512], False),
                ):
                    nc.tensor.matmul(
                        out,
                        lhsT=sel,
                        rhs=rhs,
                        start=st,
                        stop=last,
                        perf_mode=DR,
                        skip_group_check=True,
                    )

            def emit_and(esc):
                # mask: one int16 AND over a pair's fp8 esc
                kk = and_pending[1]
                v16 = esc[:].rearrange("p a (h q) -> p a h q", h=2).bitcast(i16)
                msl = mask_sb[kk // 2][:, :, ds(qw * 256, 256)]
                m16b = msl.unsqueeze(2).to_broadcast([P, 2, 2, 256])
                nc.vector.tensor_tensor(v16, v16, m16b, band)

            and_pending = None
            za_pending = []
            esc8 = None
            for kt in range(16):
                par = kt % 2
                if hp == 0 and qw == 0:
                    emit_vproj(kt)
                if kt % 4 == 1:
                    ci = qw * 4 + kt // 4
                    if ci % 2 == 0 and ci // 2 < len(next_chunks):
                        emit_proj_chunk(*next_chunks[ci // 2])
                if par == 0:
                    esc8 = work.tile([P, 2, 1024], f8e4, tag="esc", bufs=6, name="esc8")
                ps = pp.tile([P, 1024], f32, tag="sc", bufs=2, name="ps")
                nc.tensor.matmul(
                    ps[:, 0:512],
                    lhsT=qkT_sb[0:64, 4 + hp, ts(kt, P)],
                    rhs=qkT_sb[0:64, hp, ds(qw * 512, 512)],
                    start=True,
                    stop=True,
                    tile_position=(0, 0),
                )
                nc.tensor.matmul(
                    ps[:, 512:1024],
                    lhsT=qkT_sb[64:P, 4 + hp, ts(kt, P)],
                    rhs=qkT_sb[64:P, hp, ds(qw * 512, 512)],
                    start=True,
                    stop=True,
                    tile_position=(64, 0),
                )
                # Z/A of an older pair behind this k-tile's scores
                if par == 0 and len(za_pending) > 0 and kt >= 4:
                    emit_za(*za_pending.pop(0), False)
                # head A + head B upper: ACT true exp; head B q-tail: DVE
                # Schraudolph (int8 RNE -> e5m2 bits)
                nc.scalar.activation(esc8[:, par, 0:SQ], ps[:, 0:SQ], Exp, bias=biasC0[:], scale=1.0 / A8)
                nc.vector.tensor_scalar(
                    esc8[:, par, SQ:1024].bitcast(i8), ps[:, SQ:1024], 0.5, SCH_B5,
                    mybir.AluOpType.mult, add,
                )
                if par == 1:
                    if and_pending is not None:
                        emit_and(and_pending[0])
                        za_pending.append((and_pending[1] - 1, and_pending[0]))
                    and_pending = (esc8, kt)
            emit_and(and_pending[0])
            za_pending.append((and_pending[1] - 1, and_pending[0]))
            for i, p in enumerate(za_pending):
                emit_za(*p, i == len(za_pending) - 1)
            stq = work.tile([2, 2, 512], f32, tag="stq", bufs=2, name="stq")
            nc.vector.tensor_copy(stq[:, 0, :], zq[0][0:2, :])
            nc.vector.tensor_copy(stq[:, 1, :], zq[1][0:2, :])
            for j, h in enumerate((hA, hB)):
                nc.sync.dma_start(
                    za[:, h, ds(qw * 512, 512)],
                    stq[:, j, :],
                )


def _get_nc():
    if "nc" not in _CACHE:
        _CACHE["nc"] = _build_bass()
    return _CACHE["nc"]


def _pack_128(a):
    """[R, F] row-major -> [128, R//128, F] with [p, c, f] = a[128c+p, f]."""
    r, f = a.shape
    return np.ascontiguousarray(a.reshape(r // P, P, f).transpose(1, 0, 2))


def _to_e4(a):
    import jax.numpy as jnp

    return np.asarray(jnp.asarray(a, dtype=jnp.float8_e4m3))


def make_in_maps(x, att_mask, W_qk, W_v):
    Wq = np.asarray(W_qk[:, : N_HEADS * HEAD_DIM]) * (A8 / np.sqrt(HEAD_DIM) * XSCALE)
    Wk = np.asarray(W_qk[:, N_HEADS * HEAD_DIM :]) * XSCALE
    Wv = np.asarray(W_v) * XSCALE
    in_maps = []
    for c in range(NCORES):
        b, g = divmod(c, 2)
        if g == 0:
            xT_b = _pack_128(_to_e4(np.asarray(x[b]).T / XSCALE))
            liveT = (~np.asarray(att_mask[b])).T  # [k, q]
            m16 = (
                liveT[:, 0::2] * 0x00FF + liveT[:, 1::2] * 0xFF00
            ).astype(np.uint16).view(np.int16)
            maskT16_b = _pack_128(m16)
        wc = np.concatenate(
            [
                Wq[:, 512 * g : 512 * (g + 1)],
                Wk[:, 512 * g : 512 * (g + 1)],
                Wv[:, HPC * g : HPC * (g + 1)],
                np.zeros((HID, 8), np.float32),
            ],
            axis=1,
        )
        in_maps.append({"xT": xT_b, "maskT16": maskT16_b, "w": _pack_128(_to_e4(wc))})
    return in_maps


def _combine(za_list, att_mask):
    bs = att_mask.shape[0]
    attended = np.zeros((bs, QS), np.float64)
    for c in range(NCORES):
        b = c // 2
        z = za_list[c][0].astype(np.float64)  # [8, QS]
        a = za_list[c][1].astype(np.float64)
        attended[b] += (a / z).sum(axis=0)
    pm = np.asarray(att_mask[:, -1])
    o = np.where(pm, NEG, attended)
    out = np.where(o >= 0, 1.0 / (1.0 + np.exp(-np.clip(o, 0, None))),
                   np.exp(np.clip(o, None, 0)) / (1.0 + np.exp(np.clip(o, None, 0))))
    return out[..., None].astype(np.float32)


def kernel(x, att_mask, W_qk, W_v):
    from concourse.bass_utils import run_bass_kernel_spmd

    nc = _get_nc()
    in_maps = make_in_maps(x, att_mask, W_qk, W_v)
    res = run_bass_kernel_spmd(nc, in_maps, core_ids=list(range(NCORES)))
    _CACHE["last_results"] = res
    za_list = [r["za"] for r in res.results]
    return _combine(za_list, np.asarray(att_mask))


def _make_runner(nc):
    """Cached-jit SPMD runner modeled on bass2jax.run_bass_via_pjrt (no
    donation so device-resident inputs survive across calls)."""
    import jax
    from jax.sharding import Mesh, PartitionSpec
    from jax.experimental.shard_map import shard_map

    import concourse.mybir as mybir
    from concourse import bass2jax

    bass2jax.install_neuronx_cc_hook()
    partition_name = nc.partition_id_tensor.name if nc.partition_id_tensor else None
    in_names, out_names, out_avals, zero_outs = [], [], [], []
    for alloc in nc.m.functions[0].allocations:
        if not isinstance(alloc, mybir.MemoryLocationSet):
            continue
        name = alloc.memorylocations[0].name
        if alloc.kind == "ExternalInput":
            if name != partition_name:
                in_names.append(name)
        elif alloc.kind == "ExternalOutput":
            shape = tuple(alloc.tensor_shape)
            dtype = mybir.dt.np(alloc.dtype)
            out_names.append(name)
            out_avals.append(jax.core.ShapedArray(shape, dtype))
            zero_outs.append(np.zeros(shape, dtype))
    n_params = len(in_names)
    all_in_names = in_names + out_names
    if partition_name is not None:
        all_in_names.append(partition_name)

    def _body(*args):
        operands = list(args)
        if partition_name is not None:
            operands.append(bass2jax.partition_id_tensor())
        outs = bass2jax._bass_exec_p.bind(
            *operands,
            out_avals=tuple(out_avals),
            in_names=tuple(all_in_names),
            out_names=tuple(out_names),
            lowering_input_output_aliases=(),
            sim_require_finite=True,
            sim_require_nnan=True,
            nc=nc,
        )
        return tuple(outs)

    devices = jax.devices()[:NCORES]
    mesh = Mesh(np.asarray(devices), ("core",))
    in_specs = (PartitionSpec("core"),) * (n_params + len(out_names))
    out_specs = (PartitionSpec("core"),) * len(out_names)
    sharded = jax.jit(
        shard_map(_body, mesh=mesh, in_specs=in_specs, out_specs=out_specs, check_rep=False),
        keep_unused=True,
    )

    def put(in_maps):
        concat_in = [
            np.concatenate([np.asarray(in_maps[c][nm]) for c in range(NCORES)], axis=0)
            for nm in in_names
        ]
        concat_zero = [np.zeros((NCORES * z.shape[0], *z.shape[1:]), z.dtype) for z in zero_outs]
        return [jax.device_put(a) for a in concat_in + concat_zero]

    def run(dev_args):
        outs = sharded(*dev_args)
        jax.block_until_ready(outs)
        return outs

    def unpack(outs):
        return [
            {nm: np.asarray(outs[i]).reshape(NCORES, *out_avals[i].shape)[c]
             for i, nm in enumerate(out_names)}
            for c in range(NCORES)
        ]

    return put, run, unpack


def bench(x, att_mask, W_qk, W_v, k=1025, reps=4):
    """Estimate per-iteration device time via For_i loop-count delta."""
    import time

    in_maps = make_in_maps(x, att_mask, W_qk, W_v)
    walls = {}
    for iters in (1, k):
        nc = _build_bass(loop_iters=iters)
        put, run, unpack = _make_runner(nc)
        dev_args = put(in_maps)
        run(dev_args)  # warm (compile)
        ts = []
        for _ in range(reps):
            t0 = time.monotonic()
            run(dev_args)
            ts.append(time.monotonic() - t0)
        walls[iters] = ts
        print(f"iters={iters}: walls {' '.join(f'{t*1e3:.1f}ms' for t in ts)}")
    per_iter = (min(walls[k]) - min(walls[1])) / (k - 1)
    print(f"per-iteration device time: {per_iter*1e6:.1f} us")
    print(f"HW exec time: {per_iter*1e9:.0f} ns")
    return per_iter
